# revision 16
# baseline (speedup 1.0000x reference)
"""Trainium2 Bass kernel for nn_Block_47545287967557 (dense_cnn).

The reference module, simplified:
  - dead avgpool->linear->relu path (result unused)
  - sum over K=4 conv branches == ONE 3x3 VALID conv with weights Wc.sum(0)
    and bias bc.sum(0):  O[b,co,y,x] = sum_{ci,dy,dx} Weff[co,ci,dy,dx] *
    X[b,ci,y+dy,x+dx] + beff[co]
  X: [32,3,512,512] fp32 -> O: [32,3,510,510] fp32.

Strategy: pure data-parallel over batch across 8 NeuronCores (4 images each).
Per core the conv runs on the tensor engine as block-banded matmuls:
  contraction K = (c_in, yi) packed into 126 partitions (42-row y window),
  output M = (c_out, yo) packed into 120 partitions (+8 zero pad to 128 for
  FWL), moving N = 510 x positions; one matmul per dx shift (3,
  PSUM-accumulated). 13 y-blocks per image (y0 = 0,40,...,440,470; the last
  overlaps rows 470..479 with identical values). X is DMA'd as fp32 and cast
  to bf16 on VectorE (bf16 matmuls stream 1 col/cycle with pipelined
  LDWEIGHTS; fp32/fp32r pay a serialized ~180ns weight load per matmul).
  Bias is fused into the PSUM->SBUF copy on ScalarE (Identity+bias).

DMA layout: HBM DMA efficiency on trn2 is descriptor-size bound (measured:
2KB descs -> ~155 GB/s, >=8KB descs -> ~350-386 GB/s; the HWDGE ring feeds
~1 desc/13ns). So the host shards X directly into the matmul layout
XP[img, (c,yi), b, x] (the overlap-window gather is part of sharding), and
the device writes output partition-major OUT[img, (c,yo), b, x]; the host
inverts that layout while unsharding. Every DMA then moves >=8KB contiguous
per partition (~1000 descriptors/core instead of ~14000).
"""

import sys

sys.path.insert(0, "/opt/trn_rl_repo")

import numpy as np

N_CORES = 8
B_PER_CORE = 4
C = 3
H = W = 512
OH = OW = 510
NBLK = 13
KP = C * 42    # 126 contraction partitions
MP = C * 40    # 120 live output partitions
MPAD = 128     # stationary columns padded for FWL
CHUNKS = [(0, 7), (7, 13)]  # DMA/cast/store granularity (descs stay >=12KB)

_CACHE = {}


def _build_weights(Wc, bc):
    import ml_dtypes

    Weff = np.asarray(Wc, dtype=np.float32).sum(axis=0)  # [co, ci, dy, dx]
    beff = np.asarray(bc, dtype=np.float32).sum(axis=0)  # [co]
    S = np.zeros((3, KP, MPAD), dtype=np.float32)
    for dx in range(3):
        for c_in in range(C):
            for c_out in range(C):
                for yo in range(40):
                    for dy in range(3):
                        S[dx, c_in * 42 + yo + dy, c_out * 40 + yo] = Weff[c_out, c_in, dy, dx]
    Sb = S.astype(np.float16)
    biasv = np.repeat(beff, 40).reshape(MP, 1).astype(np.float32)
    return Sb, biasv


def _build_program():
    import concourse.bass as bass
    import concourse.mybir as mybir
    import concourse.tile as tile
    from concourse import bacc

    nc = bacc.Bacc("TRN2", target_bir_lowering=False, debug=False)

    XS = nc.dram_tensor("XS", [B_PER_CORE, KP, NBLK, W], mybir.dt.float32, kind="ExternalInput")
    SMAT = nc.dram_tensor("SMAT", [3, KP, MPAD], mybir.dt.float16, kind="ExternalInput")
    BIASV = nc.dram_tensor("BIASV", [MP, 1], mybir.dt.float32, kind="ExternalInput")
    OUT = nc.dram_tensor("OUT", [B_PER_CORE, MP, NBLK, OW], mybir.dt.float16, kind="ExternalOutput")

    f32 = mybir.dt.float32
    f16 = mybir.dt.float16
    ident = mybir.ActivationFunctionType.Identity

    with tile.TileContext(nc) as tc:
        with (
            tc.tile_pool(name="consts", bufs=1) as consts,
            tc.tile_pool(name="xs", bufs=3) as xpool,
            tc.tile_pool(name="xb", bufs=2) as bpool,
            tc.tile_pool(name="os", bufs=3) as opool,
            tc.tile_pool(name="ps", bufs=8, space=bass.MemorySpace.PSUM) as ppool,
        ):
            smat_t = []
            for d in range(3):
                st = consts.tile([KP, MPAD], f16, tag=f"smat{d}")
                nc.gpsimd.dma_start(out=st[:], in_=SMAT.ap()[d])
                smat_t.append(st)
            bias_t = consts.tile([MP, 1], f32, tag="biasv")
            nc.gpsimd.dma_start(out=bias_t[:], in_=BIASV.ap())

            for img in range(B_PER_CORE):
                xt = xpool.tile([KP, NBLK, W], f32)
                xb = bpool.tile([KP, NBLK, W], f16)
                ot = opool.tile([MP, NBLK, OW], f16)
                for b0, b1 in CHUNKS:
                    nc.sync.dma_start(out=xt[:, b0:b1, :], in_=XS.ap()[img, :, b0:b1, :])
                    nc.vector.tensor_copy(xb[:, b0:b1, :], xt[:, b0:b1, :])
                    # dx-major: consecutive matmuls share one stationary ->
                    # 3 weight loads per chunk instead of 3 per block
                    pts = [ppool.tile([MPAD, OW], f32, name="pt", tag="pt") for _ in range(b0, b1)]
                    for dx in range(3):
                        for j, b in enumerate(range(b0, b1)):
                            nc.tensor.matmul(
                                pts[j][:],
                                smat_t[dx][:],
                                xb[:, b, dx:dx + OW],
                                start=(dx == 0),
                                stop=(dx == 2),
                            )
                    for j, b in enumerate(range(b0, b1)):
                        if b % 2 == 0:
                            nc.scalar.activation(
                                ot[:, b, :], pts[j][0:MP, :], ident, bias=bias_t[:, 0:1], scale=1.0
                            )
                        else:
                            nc.vector.tensor_scalar(
                                ot[:, b, :], pts[j][0:MP, :], bias_t[:, 0:1], None,
                                op0=mybir.AluOpType.add,
                            )
                    nc.scalar.dma_start(out=OUT.ap()[img, :, b0:b1, :], in_=ot[:, b0:b1, :])

    nc.compile()
    return nc


def _get_nc():
    if "nc" not in _CACHE:
        _CACHE["nc"] = _build_program()
    return _CACHE["nc"]


def run_spmd(in_maps, **kwargs):
    from concourse.bass_utils import run_bass_kernel_spmd

    nc = _get_nc()
    return run_bass_kernel_spmd(nc, in_maps, list(range(N_CORES)), **kwargs)


def make_in_maps(X, Wc, bc):
    X = np.ascontiguousarray(np.asarray(X, dtype=np.float32))
    Sb, biasv = _build_weights(Wc, bc)

    # overlap-window shard: XP[core, img, c*42+yi, b, x] = X[4*core+img, c, y0(b)+yi, x]
    Xr = X.reshape(N_CORES, B_PER_CORE, C, H, W)
    XP = np.empty((N_CORES, B_PER_CORE, C, 42, NBLK, W), dtype=np.float32)
    s = Xr.strides
    win = np.lib.stride_tricks.as_strided(
        Xr, shape=(N_CORES, B_PER_CORE, C, 12, 42, W),
        strides=(s[0], s[1], s[2], 40 * s[3], s[3], s[4]))
    XP[:, :, :, :, 0:12, :] = win.transpose(0, 1, 2, 4, 3, 5)
    XP[:, :, :, :, 12, :] = Xr[:, :, :, 470:512, :]
    XP = XP.reshape(N_CORES, B_PER_CORE, KP, NBLK, W)

    return [
        {"XS": XP[i], "SMAT": Sb, "BIASV": biasv}
        for i in range(N_CORES)
    ]


def gather_output(res):
    """[core][img, (c,yo), b, x] -> [32, 3, 510, 510]"""
    OUTP = np.stack([res.results[i]["OUT"] for i in range(N_CORES)]).astype(np.float32)
    R = OUTP.reshape(N_CORES, B_PER_CORE, C, 40, NBLK, OW)
    O = np.empty((N_CORES, B_PER_CORE, C, OH, OW), dtype=np.float32)
    O[:, :, :, 0:480, :] = (
        R[:, :, :, :, 0:12, :].transpose(0, 1, 2, 4, 3, 5).reshape(N_CORES, B_PER_CORE, C, 480, OW)
    )
    O[:, :, :, 480:OH, :] = R[:, :, :, 10:40, 12, :]
    return O.reshape(N_CORES * B_PER_CORE, C, OH, OW)


def kernel(X, Wc, bc, linW, linb):
    res = run_spmd(make_in_maps(X, Wc, bc))
    return gather_output(res)


# revision 17
# speedup vs baseline: 1.0561x; 1.0561x over previous
"""Trainium2 Bass kernel for nn_Block_47545287967557 (dense_cnn).

The reference module, simplified:
  - dead avgpool->linear->relu path (result unused)
  - sum over K=4 conv branches == ONE 3x3 VALID conv with weights Wc.sum(0)
    and bias bc.sum(0):  O[b,co,y,x] = sum_{ci,dy,dx} Weff[co,ci,dy,dx] *
    X[b,ci,y+dy,x+dx] + beff[co]
  X: [32,3,512,512] fp32 -> O: [32,3,510,510] fp32.

Strategy: pure data-parallel over batch across 8 NeuronCores (4 images each).
Per core the conv runs on the tensor engine as block-banded matmuls:
  contraction K = (c_in, yi) packed into 126 partitions (42-row y window),
  output M = (c_out, yo) packed into 120 partitions (+8 zero pad to 128 for
  FWL), moving N = 510 x positions; one matmul per dx shift (3,
  PSUM-accumulated). 13 y-blocks per image (y0 = 0,40,...,440,470; the last
  overlaps rows 470..479 with identical values). X is DMA'd as fp32 and cast
  to bf16 on VectorE (bf16 matmuls stream 1 col/cycle with pipelined
  LDWEIGHTS; fp32/fp32r pay a serialized ~180ns weight load per matmul).
  Bias is fused into the PSUM->SBUF copy on ScalarE (Identity+bias), which
  stores the output as fp16 (the host upcasts to fp32 while unsharding; the
  values already carry fp16-input precision, measured rel err ~6e-4).

DMA layout: HBM DMA efficiency on trn2 is descriptor-size bound (measured:
2KB descs -> ~155 GB/s, >=8KB descs -> ~350-386 GB/s; the HWDGE ring feeds
~1 desc/13ns). So the host shards X directly into the matmul layout
XP[img, (c,yi), b, x] (the overlap-window gather is part of sharding), and
the device writes output partition-major OUT[img, (c,yo), b, x]; the host
inverts that layout while unsharding. Every DMA then moves >=8KB contiguous
per partition (~1000 descriptors/core instead of ~14000).
"""

import sys

sys.path.insert(0, "/opt/trn_rl_repo")

import numpy as np

N_CORES = 8
B_PER_CORE = 4
C = 3
H = W = 512
OH = OW = 510
NBLK = 13
KP = C * 42    # 126 contraction partitions
MP = C * 40    # 120 live output partitions
MPAD = 128     # stationary columns padded for FWL
CHUNKS = [(0, 7), (7, 13)]  # DMA/cast/store granularity (descs stay >=12KB)

_CACHE = {}


def _build_weights(Wc, bc):
    import ml_dtypes

    Weff = np.asarray(Wc, dtype=np.float32).sum(axis=0)  # [co, ci, dy, dx]
    beff = np.asarray(bc, dtype=np.float32).sum(axis=0)  # [co]
    S = np.zeros((3, KP, MPAD), dtype=np.float32)
    for dx in range(3):
        for c_in in range(C):
            for c_out in range(C):
                for yo in range(40):
                    for dy in range(3):
                        S[dx, c_in * 42 + yo + dy, c_out * 40 + yo] = Weff[c_out, c_in, dy, dx]
    Sb = S.astype(np.float16)
    biasv = np.repeat(beff, 40).reshape(MP, 1).astype(np.float32)
    return Sb, biasv


def _build_program():
    import concourse.bass as bass
    import concourse.mybir as mybir
    import concourse.tile as tile
    from concourse import bacc

    nc = bacc.Bacc("TRN2", target_bir_lowering=False, debug=False)

    XS = nc.dram_tensor("XS", [B_PER_CORE, KP, NBLK, W], mybir.dt.float32, kind="ExternalInput")
    SMAT = nc.dram_tensor("SMAT", [3, KP, MPAD], mybir.dt.float16, kind="ExternalInput")
    BIASV = nc.dram_tensor("BIASV", [MP, 1], mybir.dt.float32, kind="ExternalInput")
    OUT = nc.dram_tensor("OUT", [B_PER_CORE, MP, NBLK, OW], mybir.dt.float16, kind="ExternalOutput")

    f32 = mybir.dt.float32
    f16 = mybir.dt.float16
    ident = mybir.ActivationFunctionType.Identity

    with tile.TileContext(nc) as tc:
        with (
            tc.tile_pool(name="consts", bufs=1) as consts,
            tc.tile_pool(name="xs", bufs=3) as xpool,
            tc.tile_pool(name="xb", bufs=2) as bpool,
            tc.tile_pool(name="os", bufs=3) as opool,
            tc.tile_pool(name="ps", bufs=8, space=bass.MemorySpace.PSUM) as ppool,
        ):
            smat_t = []
            for d in range(3):
                st = consts.tile([KP, MPAD], f16, tag=f"smat{d}")
                nc.sync.dma_start(out=st[:], in_=SMAT.ap()[d])
                smat_t.append(st)
            bias_t = consts.tile([MP, 1], f32, tag="biasv")
            nc.sync.dma_start(out=bias_t[:], in_=BIASV.ap())

            for img in range(B_PER_CORE):
                xt = xpool.tile([KP, NBLK, W], f32)
                xb = bpool.tile([KP, NBLK, W], f16)
                ot = opool.tile([MP, NBLK, OW], f16)
                for b0, b1 in CHUNKS:
                    nc.sync.dma_start(out=xt[:, b0:b1, :], in_=XS.ap()[img, :, b0:b1, :])
                    nc.vector.tensor_copy(xb[:, b0:b1, :], xt[:, b0:b1, :])
                    for b in range(b0, b1):
                        pt = ppool.tile([MPAD, OW], f32)
                        for dx in range(3):
                            nc.tensor.matmul(
                                pt[:],
                                smat_t[dx][:],
                                xb[:, b, dx:dx + OW],
                                start=(dx == 0),
                                stop=(dx == 2),
                            )
                        nc.scalar.activation(
                            ot[:, b, :], pt[0:MP, :], ident, bias=bias_t[:, 0:1], scale=1.0
                        )
                    nc.scalar.dma_start(out=OUT.ap()[img, :, b0:b1, :], in_=ot[:, b0:b1, :])

    nc.compile()
    return nc


def _get_nc():
    if "nc" not in _CACHE:
        _CACHE["nc"] = _build_program()
    return _CACHE["nc"]


def run_spmd(in_maps, **kwargs):
    from concourse.bass_utils import run_bass_kernel_spmd

    nc = _get_nc()
    return run_bass_kernel_spmd(nc, in_maps, list(range(N_CORES)), **kwargs)


def make_in_maps(X, Wc, bc):
    X = np.ascontiguousarray(np.asarray(X, dtype=np.float32))
    Sb, biasv = _build_weights(Wc, bc)

    # overlap-window shard: XP[core, img, c*42+yi, b, x] = X[4*core+img, c, y0(b)+yi, x]
    Xr = X.reshape(N_CORES, B_PER_CORE, C, H, W)
    XP = np.empty((N_CORES, B_PER_CORE, C, 42, NBLK, W), dtype=np.float32)
    s = Xr.strides
    win = np.lib.stride_tricks.as_strided(
        Xr, shape=(N_CORES, B_PER_CORE, C, 12, 42, W),
        strides=(s[0], s[1], s[2], 40 * s[3], s[3], s[4]))
    XP[:, :, :, :, 0:12, :] = win.transpose(0, 1, 2, 4, 3, 5)
    XP[:, :, :, :, 12, :] = Xr[:, :, :, 470:512, :]
    XP = XP.reshape(N_CORES, B_PER_CORE, KP, NBLK, W)

    return [
        {"XS": XP[i], "SMAT": Sb, "BIASV": biasv}
        for i in range(N_CORES)
    ]


def gather_output(res):
    """[core][img, (c,yo), b, x] -> [32, 3, 510, 510]"""
    OUTP = np.stack([res.results[i]["OUT"] for i in range(N_CORES)]).astype(np.float32)
    R = OUTP.reshape(N_CORES, B_PER_CORE, C, 40, NBLK, OW)
    O = np.empty((N_CORES, B_PER_CORE, C, OH, OW), dtype=np.float32)
    O[:, :, :, 0:480, :] = (
        R[:, :, :, :, 0:12, :].transpose(0, 1, 2, 4, 3, 5).reshape(N_CORES, B_PER_CORE, C, 480, OW)
    )
    O[:, :, :, 480:OH, :] = R[:, :, :, 10:40, 12, :]
    return O.reshape(N_CORES * B_PER_CORE, C, OH, OW)


def kernel(X, Wc, bc, linW, linb):
    res = run_spmd(make_in_maps(X, Wc, bc))
    return gather_output(res)
